# revision 32
# baseline (speedup 1.0000x reference)
"""Trainium2 Bass kernel for nn_MaskedSelfAttention (causal, QK rms-norm).

Sharding: 8 cores = 2 (batch) x 4 (head groups of 4 heads).
Each core computes qkv projection for its heads, causal attention
(no max subtraction -- scores are bounded by rms norm), and a partial FC
output over its heads' feature slice. Host sums the 4 partials per batch.

v4: bf16 matmuls + storage (fp32 PSUM accumulation, fp32 denominators).
Software-pipelined phase A (PE transposes deferred one iteration), phase B
pipelined S -> exp -> PV with exp split between ACT (exact) and DVE (int16
Schraudolph bit-trick). Denominators leave PSUM once and are gathered into
head-major layout by tiny SBUF-SBUF DMAs (no PE transposes). The FC phase
is fused into the attention loop: each 512-column group is normalized and
projected while later attention groups still run, so the PE never drains.
bf16 output, host sums partials.

Self-contained: hardcodes shapes from the problem spec.
"""

import numpy as np
import ml_dtypes

import concourse.bacc as bacc
import concourse.mybir as mybir
import concourse.tile as tile
from concourse.bass_utils import run_bass_kernel_spmd

B, L, D = 2, 2048, 1024
DH = 64
NH = D // DH            # 16 heads total
P = 128
NHC = 4                 # heads per core
E3 = 3 * NHC * DH       # 768 qkv rows per core
LB = L // P             # 16 l-blocks
KB = D // P             # 8 contraction blocks
EPS = 1e-5
F32 = mybir.dt.float32
F32R = mybir.dt.float32r
BF16 = mybir.dt.bfloat16
I16 = mybir.dt.int16

# Schraudolph exp for bf16: exp(x) ~= bitcast_bf16(int16(x*A + Bc))
SCH_A = 128.0 / float(np.log(2.0))
SCH_B = 16256.0 - 4.5

BF = ml_dtypes.bfloat16


def R(ap):
    return ap.bitcast(F32R)


FX = mybir.ActivationFunctionType
MULT = mybir.AluOpType.mult
ADD = mybir.AluOpType.add

_CACHE = {}


def _build_nc():
    nc = bacc.Bacc("TRN2", target_bir_lowering=False, debug=False)

    xT = nc.dram_tensor("xT", (D, L), BF16, kind="ExternalInput").ap()
    wqkvT = nc.dram_tensor("wqkvT", (D, E3), BF16, kind="ExternalInput").ap()
    wfcT = nc.dram_tensor("wfcT", (NHC * DH, D), BF16, kind="ExternalInput").ap()
    triu = nc.dram_tensor("triu", (P, P), BF16, kind="ExternalInput").ap()
    sel65 = nc.dram_tensor("sel65", (DH + 1, P), F32, kind="ExternalInput").ap()
    identb = nc.dram_tensor("identb", (P, P), BF16, kind="ExternalInput").ap()
    wqk = nc.dram_tensor("wqk", (P, 1), F32, kind="ExternalInput").ap()
    outp = nc.dram_tensor("outp", (L, D), BF16, kind="ExternalOutput").ap()

    with tile.TileContext(nc) as tc:
        with (
            tc.tile_pool(name="cpool", bufs=1) as cpool,
            tc.tile_pool(name="wpool", bufs=1) as wpool,
            tc.tile_pool(name="ppool", bufs=1) as ppool,
            tc.tile_pool(name="xpool", bufs=3) as xpool,
            tc.tile_pool(name="work", bufs=8) as work,
            tc.tile_pool(name="qnpool", bufs=3) as qnpool,
            tc.tile_pool(name="ptpool", bufs=4) as ptpool,
            tc.tile_pool(name="dnpool", bufs=2) as dnpool,
            tc.tile_pool(name="opool", bufs=4) as opool,
        ):
            xT_r = xT.rearrange("(ko p) l -> p ko l", p=P)
            wqkvT_r = wqkvT.rearrange("(ko p) e -> p ko e", p=P)

            # get x block 0 and the first weight chunk moving before anything
            # else so the PE can start within a few us
            xc0 = xpool.tile([P, KB, P], BF16, tag="xc", name="xc_0")
            nc.sync.dma_start(xc0, xT_r[:, :, 0:P])
            wqkv_sb = wpool.tile([P, KB, E3], BF16)
            for k in range(KB):
                nc.sync.dma_start(wqkv_sb[:, k : k + 1, :], wqkvT_r[:, k : k + 1, :])

            triu_sb = cpool.tile([P, P], BF16)
            nc.gpsimd.dma_start(triu_sb, triu)
            identb_sb = cpool.tile([P, P], BF16)
            nc.gpsimd.dma_start(identb_sb, identb)
            wqk_sb = cpool.tile([P, 1], F32)
            nc.gpsimd.dma_start(wqk_sb, wqk)
            sel65_sb = cpool.tile([DH + 1, P], F32R)
            nc.gpsimd.dma_start(sel65_sb, R(sel65))
            biasq = cpool.tile([P, 1], F32)
            nc.vector.memset(biasq, DH * EPS)

            wfc_sb = wpool.tile([P, 2, D], BF16)
            nc.gpsimd.dma_start(wfc_sb, wfcT.rearrange("(g p) e -> p g e", p=P))

            # persistent activations (per-partition bytes in comments)
            qT = ppool.tile([P, 2, L], BF16)              # 8KB [dh-pair, hp, l]
            kT = ppool.tile([P, 2, L], BF16)              # 8KB
            vext = ppool.tile([P, LB, NHC, DH + 1], BF16)  # 8.1KB, col DH = ones
            oText = ppool.tile([P, 2, L], BF16)           # 8KB O^T, normed in place


            onesb = cpool.tile([P, 1], BF16)
            nc.vector.memset(onesb, 1.0)
            nc.vector.tensor_copy(
                vext[:, :, :, DH : DH + 1],
                onesb[:, :, None, None].to_broadcast((P, LB, NHC, 1)),
            )

            # ---- Phase A: qkv projection (l,e') + rms norm + transpose q,k ----
            qn_tiles = [None] * LB

            def emit_transposes(m, psT):
                qn = qn_tiles[m]
                for g in range(4):  # blocks: 0,1 -> qT; 2,3 -> kT
                    tp = psT.tile([P, P], BF16, tag="tp", name=f"tp_{m}_{g}")
                    nc.tensor.transpose(tp, qn[:, g * P : (g + 1) * P], identb_sb)
                    if g < 2:
                        nc.vector.tensor_copy(qT[:, g, m * P : (m + 1) * P], tp)
                    else:
                        # fold q_norm_w*k_norm_w into kT during the copy
                        nc.scalar.activation(
                            kT[:, g % 2, m * P : (m + 1) * P], tp, FX.Copy,
                            scale=wqk_sb[:, :],
                        )

            with (
                tc.tile_pool(name="psA", bufs=2, space="PSUM") as psA,
                tc.tile_pool(name="psT", bufs=3, space="PSUM") as psT,
            ):
                for m in range(LB):
                    if m == 0:
                        xc = xc0
                    else:
                        xc = xpool.tile([P, KB, P], BF16, tag="xc", name=f"xc_{m}")
                        nc.sync.dma_start(xc, xT_r[:, :, m * P : (m + 1) * P])
                    ps = psA.tile([P, 2 * NHC * DH], F32, tag="qkps", bufs=3,
                                  name=f"qkps_{m}")
                    psv = psA.tile([P, NHC * DH], F32, tag="vps", bufs=2,
                                   name=f"vps_{m}")
                    for k in range(KB):
                        nc.tensor.matmul(
                            ps, lhsT=xc[:, k, :],
                            rhs=wqkv_sb[:, k, 0 : 2 * NHC * DH],
                            start=(k == 0), stop=(k == KB - 1),
                        )
                    for k in range(KB):
                        nc.tensor.matmul(
                            psv, lhsT=xc[:, k, :],
                            rhs=wqkv_sb[:, k, 2 * NHC * DH : 3 * NHC * DH],
                            start=(k == 0), stop=(k == KB - 1),
                        )
                    # transposes of the previous iteration keep the PE busy
                    # while this iteration's norm chain runs on ACT/DVE
                    if m > 0:
                        emit_transposes(m - 1, psT)
                    nc.scalar.copy(
                        vext[:, m, :, 0:DH],
                        psv.rearrange("p (h d) -> p h d", d=DH),
                    )
                    sq = work.tile([P, 2 * NHC * DH], F32, tag="sq", name=f"sq_{m}")
                    nc.scalar.activation(sq, ps, FX.Square)
                    ssq = work.tile([P, 2 * NHC], F32, tag="ssq", name=f"ssq_{m}")
                    nc.vector.reduce_sum(
                        ssq, sq.rearrange("p (h d) -> p h d", d=DH),
                        axis=mybir.AxisListType.X,
                    )
                    rin = work.tile([P, 2 * NHC], F32, tag="rin", name=f"rin_{m}")
                    # both q,k: 1/rin = 0.125 / sqrt(mean + eps); the extra 1/64
                    # vs the reference's 1/8 sdpa scale is undone by exp(scale=8)
                    nc.scalar.activation(rin, ssq, FX.Sqrt, bias=biasq[:, :], scale=1.0)
                    inv = work.tile([P, 2 * NHC], F32, tag="inv", name=f"inv_{m}")
                    nc.vector.reciprocal(inv, rin)
                    qn = qnpool.tile([P, 2 * NHC * DH], BF16, tag="qn", name=f"qn_{m}")
                    nc.vector.tensor_tensor(
                        qn.rearrange("p (h d) -> p h d", d=DH),
                        ps.rearrange("p (h d) -> p h d", d=DH),
                        inv[:, :, None].to_broadcast((P, 2 * NHC, DH)),
                        MULT,
                    )
                    qn_tiles[m] = qn
                emit_transposes(LB - 1, psT)

            # ---- Phase B: attention + fused FC ----
            # S^T = kT.T@qT, P^T = exp, O^T += V^T@P^T; after both head-pair
            # groups of a 512-column block finish, normalize and project it.
            nexp = [0]
            recdns = {}
            with (
                tc.tile_pool(name="psS", bufs=2, space="PSUM") as psS,
                tc.tile_pool(name="psO", bufs=2, space="PSUM") as psO,
            ):
                def emit_group(c, hp):
                    oTps = psO.tile([DH + 1, 2, 512], F32, tag="oT", bufs=1,
                                    name=f"oT_{hp}_{c}")
                    nj = 4 * c + 4
                    sts = [None] * nj
                    pts = [None] * nj

                    def emit_S(j):
                        off = max(0, j * P - c * 512)
                        W = 512 - off
                        st = psS.tile([P, 2, 512], F32, tag="sT", bufs=3,
                                      name=f"sT_{hp}_{c}_{j}")
                        for h2 in range(2):
                            nc.tensor.matmul(
                                st[:, h2, 0:W],
                                lhsT=kT[h2 * DH : (h2 + 1) * DH, hp,
                                        j * P : (j + 1) * P],
                                rhs=qT[h2 * DH : (h2 + 1) * DH, hp,
                                       c * 512 + off : (c + 1) * 512],
                                start=True, stop=True,
                            )
                        sts[j] = st

                    def emit_exp(j):
                        off = max(0, j * P - c * 512)
                        W = 512 - off
                        st = sts[j]
                        diag = j >= 4 * c
                        nexp[0] += 1
                        use_dve = (not diag) and (nexp[0] % 3 == 0)
                        if use_dve:
                            pti = ptpool.tile([P, 2, 512], I16, tag="pt",
                                              name=f"pti_{hp}_{c}_{j}")
                            nc.vector.tensor_scalar(
                                pti, st, 8.0 * SCH_A, SCH_B, MULT, ADD,
                            )
                            pt = pti.bitcast(BF16)
                        else:
                            pt = ptpool.tile([P, 2, 512], BF16, tag="pt",
                                             name=f"pt_{hp}_{c}_{j}")
                            nc.scalar.activation(
                                pt[:, :, 0:W], st[:, :, 0:W], FX.Exp, scale=8.0
                            )
                        if diag:
                            nc.vector.tensor_tensor(
                                pt[:, :, 0:P], pt[:, :, 0:P],
                                triu_sb[:, None, :].to_broadcast((P, 2, P)),
                                MULT,
                            )
                        pts[j] = pt

                    def emit_PV(j):
                        off = max(0, j * P - c * 512)
                        W = 512 - off
                        pt = pts[j]
                        for h2 in range(2):
                            nc.tensor.matmul(
                                oTps[:, h2, off:512],
                                lhsT=vext[:, j, 2 * hp + h2, :],
                                rhs=pt[:, h2, 0:W],
                                start=(j == 0), stop=(j == nj - 1),
                                skip_group_check=True,
                            )

                    emit_S(0)
                    if nj > 1:
                        emit_S(1)
                    for j in range(nj):
                        emit_exp(j)
                        if j + 2 < nj:
                            emit_S(j + 2)
                        emit_PV(j)

                    # denominators (fp32): row 64 of oTps -> den4 via tiny
                    # SBUF-SBUF DMA (no PE transposes)
                    gi = 2 * c + hp
                    dn = dnpool.tile([DH + 1, 512], F32, tag="dn",
                                     name=f"dn_{hp}_{c}")
                    if gi < 2:
                        # first use of each rotating buffer: finite filler so
                        # the rb matmul's zero weights never hit junk
                        nc.vector.memset(dn, 1.0)
                    nc.vector.tensor_copy(dn[0:1, :], oTps[DH : DH + 1, 0, :])
                    nc.scalar.copy(dn[DH : DH + 1, :], oTps[DH : DH + 1, 1, :])
                    recdn = dnpool.tile([DH + 1, 512], F32R, tag="recdn",
                                        name=f"recdn_{hp}_{c}")
                    with nc.allow_low_precision(reason="f32r recip of denom"):
                        nc.vector.reciprocal(recdn, dn)
                    recdns[(c, hp)] = recdn
                    # unnormalized O^T -> SBUF bf16
                    nc.vector.tensor_copy(
                        oText[0:DH, hp, c * 512 : (c + 1) * 512],
                        oTps[0:DH, 0, :],
                    )
                    nc.scalar.copy(
                        oText[DH : 2 * DH, hp, c * 512 : (c + 1) * 512],
                        oTps[0:DH, 1, :],
                    )

                def emit_rbmult(c, hp):
                    # broadcast 1/denom over dh partitions via rank-1 matmuls,
                    # scale O^T in place
                    rbt = psS.tile([P, 2, 512], F32, tag="sT", bufs=3,
                                   name=f"rb_{hp}_{c}")
                    rb = rbt[:, 0, :]
                    nc.tensor.matmul(
                        rb, lhsT=sel65_sb, rhs=recdns[(c, hp)],
                        start=True, stop=True,
                    )
                    seg = oText[:, hp, c * 512 : (c + 1) * 512]
                    nc.vector.tensor_tensor(seg, seg, rb, MULT)

                def emit_fcblock(c):
                    for mi in range(4):
                        m = 4 * c + mi
                        fct = psS.tile([P, 2, 512], F32, tag="sT", bufs=3,
                                       name=f"fc_{m}")
                        for n in range(2):
                            for g in range(2):
                                nc.tensor.matmul(
                                    fct[:, n, :],
                                    lhsT=oText[:, g, m * P : (m + 1) * P],
                                    rhs=wfc_sb[:, g, n * 512 : (n + 1) * 512],
                                    start=(g == 0), stop=(g == 1),
                                )
                        fp = fct.rearrange("p a b -> p (a b)")
                        ot = opool.tile([P, D], BF16, tag="ot", name=f"ot_{m}")
                        # last block: ACT-heavy (DVE is the tail critical path)
                        dve = (m % 4 == 3) if c == 3 else (m % 2 == 1)
                        if dve:
                            nc.vector.tensor_copy(ot, fp)
                        else:
                            nc.scalar.copy(ot, fp)
                        nc.sync.dma_start(outp[m * P : (m + 1) * P, :], ot)

                for c in range(4):
                    emit_group(c, 0)
                    if c > 0:
                        emit_rbmult(c - 1, 1)
                        emit_fcblock(c - 1)
                    emit_group(c, 1)
                    emit_rbmult(c, 0)
                emit_rbmult(3, 1)
                emit_fcblock(3)

    nc.compile()
    return nc


def _make_in_maps(x, w_qkv, w_fc, q_norm_w, k_norm_w):
    triu_f = np.triu(np.ones((P, P))).astype(BF)
    identb_f = np.eye(P).astype(BF)
    wqk = np.tile((q_norm_w * k_norm_w).astype(np.float32), 2).reshape(P, 1)
    # sel65: rows 0 / DH pick the h2=0 / h2=1 denom rows, broadcast over dh
    sel65 = np.zeros((DH + 1, P), dtype=np.float32)
    sel65[0, 0:DH] = 1.0
    sel65[DH, DH : 2 * DH] = 1.0
    wqkvT = {}
    wfcTs = {}
    for hg in range(4):
        h0 = hg * NHC
        rows = np.concatenate(
            [
                w_qkv[h0 * DH : (h0 + NHC) * DH],
                w_qkv[D + h0 * DH : D + (h0 + NHC) * DH],
                w_qkv[2 * D + h0 * DH : 2 * D + (h0 + NHC) * DH],
            ],
            axis=0,
        )
        wqkvT[hg] = np.ascontiguousarray(rows.T).astype(BF)
        wfcTs[hg] = np.ascontiguousarray(w_fc.T[h0 * DH : (h0 + NHC) * DH]).astype(BF)
    xTs = [np.ascontiguousarray(x[b].T).astype(BF) for b in range(B)]
    in_maps = []
    for core in range(8):
        b, hg = core // 4, core % 4
        in_maps.append(
            {
                "xT": xTs[b],
                "wqkvT": wqkvT[hg],
                "wfcT": wfcTs[hg],
                "triu": triu_f,
                "identb": identb_f,
                "wqk": wqk,
                "sel65": sel65,
            }
        )
    return in_maps


def _is_causal(mask):
    idx = np.arange(mask.shape[0])
    return mask.shape == (L, L) and bool(np.all(mask == (idx[None, :] <= idx[:, None])))


def _reference_numpy(x, mask, w_qkv, w_fc, q_norm_w, k_norm_w, subset_attention_size):
    # slow but general fallback (only used if mask is not causal)
    b, l, d = x.shape
    qkv = x @ w_qkv.T
    q, k, v = np.split(qkv, 3, axis=-1)

    def heads(t):
        return t.reshape(b, l, NH, DH).transpose(0, 2, 1, 3)

    def rms(t, w):
        return t * (1.0 / np.sqrt(np.mean(t * t, -1, keepdims=True) + EPS)) * w

    q, k, v = heads(q), heads(k), heads(v)
    q, k = rms(q, q_norm_w), rms(k, k_norm_w)

    def sdpa(q, k, v, m):
        s = np.einsum("bhqd,bhkd->bhqk", q, k) / np.sqrt(DH)
        s = np.where(m[None, None], s, -1e30)
        s = s - s.max(-1, keepdims=True)
        p = np.exp(s)
        p /= p.sum(-1, keepdims=True)
        return np.einsum("bhqk,bhkd->bhqd", p, v)

    S = int(subset_attention_size) if subset_attention_size is not None else None
    if S is not None and S < l:
        o = np.concatenate(
            [
                sdpa(q[:, :, :S], k[:, :, :S], v[:, :, :S], mask[:S, :S]),
                sdpa(q[:, :, S:], k, v, mask[S:, :]),
            ],
            axis=2,
        )
    else:
        o = sdpa(q, k, v, mask)
    o = o.transpose(0, 2, 1, 3).reshape(b, l, d)
    return (o @ w_fc.T).astype(np.float32)


def kernel(**inputs):
    x = np.asarray(inputs["x"], dtype=np.float32)
    mask = np.asarray(inputs["mask"])
    w_qkv = np.asarray(inputs["w_qkv"], dtype=np.float32)
    w_fc = np.asarray(inputs["w_fc"], dtype=np.float32)
    q_norm_w = np.asarray(inputs["q_norm_w"], dtype=np.float32)
    k_norm_w = np.asarray(inputs["k_norm_w"], dtype=np.float32)

    if not _is_causal(mask):
        return _reference_numpy(
            x, mask, w_qkv, w_fc, q_norm_w, k_norm_w,
            inputs.get("subset_attention_size"),
        )

    if "nc" not in _CACHE:
        _CACHE["nc"] = _build_nc()
    nc = _CACHE["nc"]

    in_maps = _make_in_maps(x, w_qkv, w_fc, q_norm_w, k_norm_w)
    res = run_bass_kernel_spmd(nc, in_maps, core_ids=list(range(8)))
    parts = [res.results[i]["outp"] for i in range(8)]
    out = np.empty((B, L, D), dtype=np.float32)
    for b in range(B):
        acc = np.zeros((L, D), dtype=np.float32)
        for hg in range(4):
            acc += parts[b * 4 + hg].astype(np.float32)
        out[b] = acc
    return out


# revision 33
# speedup vs baseline: 1.0760x; 1.0760x over previous
"""Trainium2 Bass kernel for nn_MaskedSelfAttention (causal, QK rms-norm).

Sharding: 8 cores = 2 (batch) x 4 (head groups of 4 heads).
Each core computes qkv projection for its heads, causal attention
(no max subtraction -- scores are bounded by rms norm), and a partial FC
output over its heads' feature slice. Host sums the 4 partials per batch.

v4: bf16 matmuls + storage (fp32 PSUM accumulation, fp32 denominators).
Software-pipelined phase A (PE transposes deferred one iteration), phase B
pipelined S -> exp -> PV with exp split between ACT (exact) and DVE (int16
Schraudolph bit-trick). Denominators leave PSUM once and are gathered into
head-major layout by tiny SBUF-SBUF DMAs (no PE transposes). The FC phase
is fused into the attention loop: each 512-column group is normalized and
projected while later attention groups still run, so the PE never drains.
bf16 output, host sums partials.

Self-contained: hardcodes shapes from the problem spec.
"""

import numpy as np
import ml_dtypes

import concourse.bacc as bacc
import concourse.mybir as mybir
import concourse.tile as tile
from concourse.bass_utils import run_bass_kernel_spmd

B, L, D = 2, 2048, 1024
DH = 64
NH = D // DH            # 16 heads total
P = 128
NHC = 4                 # heads per core
E3 = 3 * NHC * DH       # 768 qkv rows per core
LB = L // P             # 16 l-blocks
KB = D // P             # 8 contraction blocks
EPS = 1e-5
F32 = mybir.dt.float32
F32R = mybir.dt.float32r
BF16 = mybir.dt.bfloat16
I16 = mybir.dt.int16

# Schraudolph exp for bf16: exp(x) ~= bitcast_bf16(int16(x*A + Bc))
SCH_A = 128.0 / float(np.log(2.0))
SCH_B = 16256.0 - 4.5

BF = ml_dtypes.bfloat16


def R(ap):
    return ap.bitcast(F32R)


FX = mybir.ActivationFunctionType
MULT = mybir.AluOpType.mult
ADD = mybir.AluOpType.add

_CACHE = {}


def _build_nc():
    nc = bacc.Bacc("TRN2", target_bir_lowering=False, debug=False)

    xT = nc.dram_tensor("xT", (D, L), BF16, kind="ExternalInput").ap()
    wqkvT = nc.dram_tensor("wqkvT", (D, E3), BF16, kind="ExternalInput").ap()
    wfcT = nc.dram_tensor("wfcT", (NHC * DH, D), BF16, kind="ExternalInput").ap()
    triu = nc.dram_tensor("triu", (P, P), BF16, kind="ExternalInput").ap()
    sel65 = nc.dram_tensor("sel65", (DH + 1, P), F32, kind="ExternalInput").ap()
    identb = nc.dram_tensor("identb", (P, P), BF16, kind="ExternalInput").ap()
    wqk = nc.dram_tensor("wqk", (P, 1), F32, kind="ExternalInput").ap()
    outp = nc.dram_tensor("outp", (L, D), BF16, kind="ExternalOutput").ap()

    with tile.TileContext(nc) as tc:
        with (
            tc.tile_pool(name="cpool", bufs=1) as cpool,
            tc.tile_pool(name="wpool", bufs=1) as wpool,
            tc.tile_pool(name="ppool", bufs=1) as ppool,
            tc.tile_pool(name="xpool", bufs=3) as xpool,
            tc.tile_pool(name="work", bufs=8) as work,
            tc.tile_pool(name="qnpool", bufs=3) as qnpool,
            tc.tile_pool(name="ptpool", bufs=4) as ptpool,
            tc.tile_pool(name="dnpool", bufs=2) as dnpool,
            tc.tile_pool(name="opool", bufs=4) as opool,
        ):
            xT_r = xT.rearrange("(ko p) l -> p ko l", p=P)
            wqkvT_r = wqkvT.rearrange("(ko p) e -> p ko e", p=P)

            # get x block 0 and the first weight chunk moving before anything
            # else so the PE can start within a few us
            xc0 = xpool.tile([P, KB, P], BF16, tag="xc", name="xc_0")
            nc.sync.dma_start(xc0, xT_r[:, :, 0:P])
            wqkv_sb = wpool.tile([P, KB, E3], BF16)
            for k in range(KB):
                nc.sync.dma_start(wqkv_sb[:, k : k + 1, :], wqkvT_r[:, k : k + 1, :])

            triu_sb = cpool.tile([P, P], BF16)
            nc.gpsimd.dma_start(triu_sb, triu)
            identb_sb = cpool.tile([P, P], BF16)
            nc.gpsimd.dma_start(identb_sb, identb)
            wqk_sb = cpool.tile([P, 1], F32)
            nc.gpsimd.dma_start(wqk_sb, wqk)
            sel65_sb = cpool.tile([DH + 1, P], F32R)
            nc.gpsimd.dma_start(sel65_sb, R(sel65))
            biasq = cpool.tile([P, 1], F32)
            nc.vector.memset(biasq, DH * EPS)

            wfc_sb = wpool.tile([P, 2, D], BF16)
            nc.gpsimd.dma_start(wfc_sb, wfcT.rearrange("(g p) e -> p g e", p=P))

            # persistent activations (per-partition bytes in comments)
            qT = ppool.tile([P, 2, L], BF16)              # 8KB [dh-pair, hp, l]
            kT = ppool.tile([P, 2, L], BF16)              # 8KB
            vext = ppool.tile([P, LB, NHC, DH + 1], BF16)  # 8.1KB, col DH = ones
            oText = ppool.tile([P, 2, L], BF16)           # 8KB O^T, normed in place


            onesb = cpool.tile([P, 1], BF16)
            nc.vector.memset(onesb, 1.0)
            nc.vector.tensor_copy(
                vext[:, :, :, DH : DH + 1],
                onesb[:, :, None, None].to_broadcast((P, LB, NHC, 1)),
            )

            # ---- Phase A: qkv projection (l,e') + rms norm + transpose q,k ----
            qn_tiles = [None] * LB

            def emit_transposes(m, psT):
                qn = qn_tiles[m]
                for g in range(4):  # blocks: 0,1 -> qT; 2,3 -> kT
                    tp = psT.tile([P, P], BF16, tag="tp", name=f"tp_{m}_{g}")
                    nc.tensor.transpose(tp, qn[:, g * P : (g + 1) * P], identb_sb)
                    if g < 2:
                        nc.vector.tensor_copy(qT[:, g, m * P : (m + 1) * P], tp)
                    else:
                        # fold q_norm_w*k_norm_w into kT during the copy
                        nc.scalar.activation(
                            kT[:, g % 2, m * P : (m + 1) * P], tp, FX.Copy,
                            scale=wqk_sb[:, :],
                        )

            with (
                tc.tile_pool(name="psA", bufs=2, space="PSUM") as psA,
                tc.tile_pool(name="psT", bufs=3, space="PSUM") as psT,
            ):
                for m in range(LB):
                    if m == 0:
                        xc = xc0
                    else:
                        xc = xpool.tile([P, KB, P], BF16, tag="xc", name=f"xc_{m}")
                        nc.sync.dma_start(xc, xT_r[:, :, m * P : (m + 1) * P])
                    ps = psA.tile([P, 2 * NHC * DH], F32, tag="qkps", bufs=3,
                                  name=f"qkps_{m}")
                    psv = psA.tile([P, NHC * DH], F32, tag="vps", bufs=2,
                                   name=f"vps_{m}")
                    for k in range(KB):
                        nc.tensor.matmul(
                            ps, lhsT=xc[:, k, :],
                            rhs=wqkv_sb[:, k, 0 : 2 * NHC * DH],
                            start=(k == 0), stop=(k == KB - 1),
                        )
                    for k in range(KB):
                        nc.tensor.matmul(
                            psv, lhsT=xc[:, k, :],
                            rhs=wqkv_sb[:, k, 2 * NHC * DH : 3 * NHC * DH],
                            start=(k == 0), stop=(k == KB - 1),
                        )
                    # transposes of the previous iteration keep the PE busy
                    # while this iteration's norm chain runs on ACT/DVE
                    if m > 0:
                        emit_transposes(m - 1, psT)
                    nc.scalar.copy(
                        vext[:, m, :, 0:DH],
                        psv.rearrange("p (h d) -> p h d", d=DH),
                    )
                    sq = work.tile([P, 2 * NHC * DH], F32, tag="sq", name=f"sq_{m}")
                    nc.scalar.activation(sq, ps, FX.Square)
                    ssq = work.tile([P, 2 * NHC], F32, tag="ssq", name=f"ssq_{m}")
                    nc.vector.reduce_sum(
                        ssq, sq.rearrange("p (h d) -> p h d", d=DH),
                        axis=mybir.AxisListType.X,
                    )
                    rin = work.tile([P, 2 * NHC], F32, tag="rin", name=f"rin_{m}")
                    # both q,k: 1/rin = 0.125 / sqrt(mean + eps); the extra 1/64
                    # vs the reference's 1/8 sdpa scale is undone by exp(scale=8)
                    nc.scalar.activation(rin, ssq, FX.Sqrt, bias=biasq[:, :], scale=1.0)
                    inv = work.tile([P, 2 * NHC], F32, tag="inv", name=f"inv_{m}")
                    nc.vector.reciprocal(inv, rin)
                    qn = qnpool.tile([P, 2 * NHC * DH], BF16, tag="qn", name=f"qn_{m}")
                    nc.vector.tensor_tensor(
                        qn.rearrange("p (h d) -> p h d", d=DH),
                        ps.rearrange("p (h d) -> p h d", d=DH),
                        inv[:, :, None].to_broadcast((P, 2 * NHC, DH)),
                        MULT,
                    )
                    qn_tiles[m] = qn
                emit_transposes(LB - 1, psT)

            # ---- Phase B: attention + fused FC ----
            # S^T = kT.T@qT, P^T = exp, O^T += V^T@P^T; after both head-pair
            # groups of a 512-column block finish, normalize and project it.
            nexp = [0]
            dns = {}
            recdns = {}
            with (
                tc.tile_pool(name="psS", bufs=2, space="PSUM") as psS,
                tc.tile_pool(name="psO", bufs=2, space="PSUM") as psO,
            ):
                def emit_group(c, hp, pending=()):
                    oTps = psO.tile([DH + 1, 2, 512], F32, tag="oT", bufs=1,
                                    name=f"oT_{hp}_{c}")
                    nj = 4 * c + 4
                    sts = [None] * nj
                    pts = [None] * nj

                    def emit_S(j):
                        off = max(0, j * P - c * 512)
                        W = 512 - off
                        st = psS.tile([P, 2, 512], F32, tag="sT", bufs=3,
                                      name=f"sT_{hp}_{c}_{j}")
                        for h2 in range(2):
                            nc.tensor.matmul(
                                st[:, h2, 0:W],
                                lhsT=kT[h2 * DH : (h2 + 1) * DH, hp,
                                        j * P : (j + 1) * P],
                                rhs=qT[h2 * DH : (h2 + 1) * DH, hp,
                                       c * 512 + off : (c + 1) * 512],
                                start=True, stop=True,
                            )
                        sts[j] = st

                    def emit_exp(j):
                        off = max(0, j * P - c * 512)
                        W = 512 - off
                        st = sts[j]
                        diag = j >= 4 * c
                        nexp[0] += 1
                        use_dve = (not diag) and (nexp[0] % 3 == 0)
                        if use_dve:
                            pti = ptpool.tile([P, 2, 512], I16, tag="pt",
                                              name=f"pti_{hp}_{c}_{j}")
                            nc.vector.tensor_scalar(
                                pti, st, 8.0 * SCH_A, SCH_B, MULT, ADD,
                            )
                            pt = pti.bitcast(BF16)
                        else:
                            pt = ptpool.tile([P, 2, 512], BF16, tag="pt",
                                             name=f"pt_{hp}_{c}_{j}")
                            nc.scalar.activation(
                                pt[:, :, 0:W], st[:, :, 0:W], FX.Exp, scale=8.0
                            )
                        if diag:
                            nc.vector.tensor_tensor(
                                pt[:, :, 0:P], pt[:, :, 0:P],
                                triu_sb[:, None, :].to_broadcast((P, 2, P)),
                                MULT,
                            )
                        pts[j] = pt

                    def emit_PV(j):
                        off = max(0, j * P - c * 512)
                        W = 512 - off
                        pt = pts[j]
                        for h2 in range(2):
                            nc.tensor.matmul(
                                oTps[:, h2, off:512],
                                lhsT=vext[:, j, 2 * hp + h2, :],
                                rhs=pt[:, h2, 0:W],
                                start=(j == 0), stop=(j == nj - 1),
                                skip_group_check=True,
                            )

                    emit_S(0)
                    if nj > 1:
                        emit_S(1)
                    for j in range(nj):
                        emit_exp(j)
                        if j == 1:
                            # previous groups' denominator reciprocals: deps
                            # resolved long ago, slot them into the DVE stream
                            for pc, php in pending:
                                emit_recip(pc, php)
                        if j + 2 < nj:
                            emit_S(j + 2)
                        emit_PV(j)
                    if nj == 1:
                        for pc, php in pending:
                            emit_recip(pc, php)

                    # denominators (fp32): row 64 of oTps -> den4 via tiny
                    # SBUF-SBUF DMA (no PE transposes)
                    gi = 2 * c + hp
                    dn = dnpool.tile([DH + 1, 512], F32, tag="dn",
                                     name=f"dn_{hp}_{c}")
                    if gi < 2:
                        # first use of each rotating buffer: finite filler so
                        # the rb matmul's zero weights never hit junk
                        nc.vector.memset(dn, 1.0)
                    nc.vector.tensor_copy(dn[0:1, :], oTps[DH : DH + 1, 0, :])
                    nc.scalar.copy(dn[DH : DH + 1, :], oTps[DH : DH + 1, 1, :])
                    dns[(c, hp)] = dn
                    # unnormalized O^T -> SBUF bf16
                    nc.vector.tensor_copy(
                        oText[0:DH, hp, c * 512 : (c + 1) * 512],
                        oTps[0:DH, 0, :],
                    )
                    nc.scalar.copy(
                        oText[DH : 2 * DH, hp, c * 512 : (c + 1) * 512],
                        oTps[0:DH, 1, :],
                    )

                def emit_recip(c, hp):
                    recdn = dnpool.tile([DH + 1, 512], F32R, tag="recdn",
                                        name=f"recdn_{hp}_{c}")
                    with nc.allow_low_precision(reason="f32r recip of denom"):
                        nc.vector.reciprocal(recdn, dns[(c, hp)])
                    recdns[(c, hp)] = recdn

                def emit_rbmult(c, hp):
                    # broadcast 1/denom over dh partitions via rank-1 matmuls,
                    # scale O^T in place
                    rbt = psS.tile([P, 2, 512], F32, tag="sT", bufs=3,
                                   name=f"rb_{hp}_{c}")
                    rb = rbt[:, 0, :]
                    nc.tensor.matmul(
                        rb, lhsT=sel65_sb, rhs=recdns[(c, hp)],
                        start=True, stop=True,
                    )
                    seg = oText[:, hp, c * 512 : (c + 1) * 512]
                    nc.vector.tensor_tensor(seg, seg, rb, MULT)

                def emit_fcblock(c):
                    for mi in range(4):
                        m = 4 * c + mi
                        fct = psS.tile([P, 2, 512], F32, tag="sT", bufs=3,
                                       name=f"fc_{m}")
                        for n in range(2):
                            for g in range(2):
                                nc.tensor.matmul(
                                    fct[:, n, :],
                                    lhsT=oText[:, g, m * P : (m + 1) * P],
                                    rhs=wfc_sb[:, g, n * 512 : (n + 1) * 512],
                                    start=(g == 0), stop=(g == 1),
                                )
                        fp = fct.rearrange("p a b -> p (a b)")
                        ot = opool.tile([P, D], BF16, tag="ot", name=f"ot_{m}")
                        # last block: ACT-heavy (DVE is the tail critical path)
                        dve = (m % 4 == 3) if c == 3 else (m % 2 == 1)
                        if dve:
                            nc.vector.tensor_copy(ot, fp)
                        else:
                            nc.scalar.copy(ot, fp)
                        nc.sync.dma_start(outp[m * P : (m + 1) * P, :], ot)

                for c in range(4):
                    emit_group(c, 0, pending=([(c - 1, 1)] if c > 0 else []))
                    if c > 0:
                        emit_rbmult(c - 1, 0)
                        emit_rbmult(c - 1, 1)
                        emit_fcblock(c - 1)
                    emit_group(c, 1, pending=[(c, 0)])
                emit_recip(3, 1)
                emit_rbmult(3, 0)
                emit_rbmult(3, 1)
                emit_fcblock(3)

    nc.compile()
    return nc


def _make_in_maps(x, w_qkv, w_fc, q_norm_w, k_norm_w):
    triu_f = np.triu(np.ones((P, P))).astype(BF)
    identb_f = np.eye(P).astype(BF)
    wqk = np.tile((q_norm_w * k_norm_w).astype(np.float32), 2).reshape(P, 1)
    # sel65: rows 0 / DH pick the h2=0 / h2=1 denom rows, broadcast over dh
    sel65 = np.zeros((DH + 1, P), dtype=np.float32)
    sel65[0, 0:DH] = 1.0
    sel65[DH, DH : 2 * DH] = 1.0
    wqkvT = {}
    wfcTs = {}
    for hg in range(4):
        h0 = hg * NHC
        rows = np.concatenate(
            [
                w_qkv[h0 * DH : (h0 + NHC) * DH],
                w_qkv[D + h0 * DH : D + (h0 + NHC) * DH],
                w_qkv[2 * D + h0 * DH : 2 * D + (h0 + NHC) * DH],
            ],
            axis=0,
        )
        wqkvT[hg] = np.ascontiguousarray(rows.T).astype(BF)
        wfcTs[hg] = np.ascontiguousarray(w_fc.T[h0 * DH : (h0 + NHC) * DH]).astype(BF)
    xTs = [np.ascontiguousarray(x[b].T).astype(BF) for b in range(B)]
    in_maps = []
    for core in range(8):
        b, hg = core // 4, core % 4
        in_maps.append(
            {
                "xT": xTs[b],
                "wqkvT": wqkvT[hg],
                "wfcT": wfcTs[hg],
                "triu": triu_f,
                "identb": identb_f,
                "wqk": wqk,
                "sel65": sel65,
            }
        )
    return in_maps


def _is_causal(mask):
    idx = np.arange(mask.shape[0])
    return mask.shape == (L, L) and bool(np.all(mask == (idx[None, :] <= idx[:, None])))


def _reference_numpy(x, mask, w_qkv, w_fc, q_norm_w, k_norm_w, subset_attention_size):
    # slow but general fallback (only used if mask is not causal)
    b, l, d = x.shape
    qkv = x @ w_qkv.T
    q, k, v = np.split(qkv, 3, axis=-1)

    def heads(t):
        return t.reshape(b, l, NH, DH).transpose(0, 2, 1, 3)

    def rms(t, w):
        return t * (1.0 / np.sqrt(np.mean(t * t, -1, keepdims=True) + EPS)) * w

    q, k, v = heads(q), heads(k), heads(v)
    q, k = rms(q, q_norm_w), rms(k, k_norm_w)

    def sdpa(q, k, v, m):
        s = np.einsum("bhqd,bhkd->bhqk", q, k) / np.sqrt(DH)
        s = np.where(m[None, None], s, -1e30)
        s = s - s.max(-1, keepdims=True)
        p = np.exp(s)
        p /= p.sum(-1, keepdims=True)
        return np.einsum("bhqk,bhkd->bhqd", p, v)

    S = int(subset_attention_size) if subset_attention_size is not None else None
    if S is not None and S < l:
        o = np.concatenate(
            [
                sdpa(q[:, :, :S], k[:, :, :S], v[:, :, :S], mask[:S, :S]),
                sdpa(q[:, :, S:], k, v, mask[S:, :]),
            ],
            axis=2,
        )
    else:
        o = sdpa(q, k, v, mask)
    o = o.transpose(0, 2, 1, 3).reshape(b, l, d)
    return (o @ w_fc.T).astype(np.float32)


def kernel(**inputs):
    x = np.asarray(inputs["x"], dtype=np.float32)
    mask = np.asarray(inputs["mask"])
    w_qkv = np.asarray(inputs["w_qkv"], dtype=np.float32)
    w_fc = np.asarray(inputs["w_fc"], dtype=np.float32)
    q_norm_w = np.asarray(inputs["q_norm_w"], dtype=np.float32)
    k_norm_w = np.asarray(inputs["k_norm_w"], dtype=np.float32)

    if not _is_causal(mask):
        return _reference_numpy(
            x, mask, w_qkv, w_fc, q_norm_w, k_norm_w,
            inputs.get("subset_attention_size"),
        )

    if "nc" not in _CACHE:
        _CACHE["nc"] = _build_nc()
    nc = _CACHE["nc"]

    in_maps = _make_in_maps(x, w_qkv, w_fc, q_norm_w, k_norm_w)
    res = run_bass_kernel_spmd(nc, in_maps, core_ids=list(range(8)))
    parts = [res.results[i]["outp"] for i in range(8)]
    out = np.empty((B, L, D), dtype=np.float32)
    for b in range(B):
        acc = np.zeros((L, D), dtype=np.float32)
        for hg in range(4):
            acc += parts[b * 4 + hg].astype(np.float32)
        out[b] = acc
    return out
